# revision 20
# baseline (speedup 1.0000x reference)
"""Multi-head attention (ESIM-style masked softmax) on 8 trn2 NeuronCores.

Sharding: core c -> (batch b = c//2, head-group g = c%2). Each core runs
batch b with 8 of the 16 heads: Q/K/V projections restricted to its 512
channels (Wq/Wk/Wv column shards), attention, and a partial output
projection with its 512 rows of Wo. Host sums the two partials per batch.

v2: attention processes HEAD PAIRS with PE-array tiling so the dh=64
matmuls stop wasting half the array:
  - scores: heads 2p/2p+1 run as concurrent row-tiled matmuls
    (tile_position (0,0)/(64,0), contraction 64 each);
  - key-mask enters as a per-partition exp bias (-18 for masked keys),
    so V needs no mask pre-scale and one [128,1024] exp covers both
    heads of a pair;
  - PV: col-tiled concurrent pair (out partitions 0-63 / 64-127 of one
    PSUM bank, M=64 each);
  - softmax denominators: M=1 matmuls against a bf16 key-mask column,
    4-way col-tiled at partition strips 0/32/64/96 of one PSUM bank.
All matmul operands bf16; accumulation fp32 in PSUM.
"""
import sys

for _p in ("/opt/trn_rl_repo",):
    if _p not in sys.path:
        sys.path.insert(0, _p)

import numpy as np

import concourse.bass as bass
import concourse.tile as tile
from concourse import mybir
from concourse.bass_utils import run_bass_kernel_spmd

# ---------------------------------------------------------------------------
# Workaround for this container's walrus build: it accepts at most ONE sem
# wait per lowered instruction. Split excess waits onto injected nops on the
# same (in-order) engine queue, and do the same for the kernel-tail drain.
# ---------------------------------------------------------------------------
import bass_rust
import concourse.tile as tile_mod
from concourse.vector_clock import ScopedClock

_MAX_WAITS = 1
_N_CARRIERS = 32
_wsplit_counter = [0]


def _patched_drain_and_barrier(self, tick_clock, wait_clock):
    nc = self.nc
    pre = [nc.sync.drain() for _ in range(_N_CARRIERS)]
    drain_inst = nc.sync.drain()
    wait_clock.add_sem_waits(
        drain_inst.ins, ScopedClock({None: tick_clock.global_clock})
    )
    si = drain_inst.ins.sync_info
    waits = list(si.on_wait) if si is not None else []
    if len(waits) > _MAX_WAITS:
        chunks = [waits[i : i + _MAX_WAITS] for i in range(0, len(waits), _MAX_WAITS)]
        *head, tail = chunks
        assert len(head) <= len(pre), f"too many drain waits: {len(waits)}"
        for inst, chunk in zip(pre, head):
            inst.ins.sync_info = bass_rust.SyncInfo(on_wait=chunk, on_update=[])
        drain_inst.ins.sync_info = bass_rust.SyncInfo(
            on_wait=tail, on_update=list(si.on_update) if si else []
        )
    nc.all_engine_barrier()
    assert self.sems is not None
    popped = nc._tile_sem_poison_stack.pop()
    assert popped is self._sem_poison
    nc.clear_and_free_semaphores(list(self.sems.allocated().values()))
    nc.all_engine_barrier()


def _split_excess_waits(nc, max_waits=_MAX_WAITS):
    n_split = 0
    for fn in nc.m.functions:
        for blk in fn.blocks:
            insts = blk.instructions
            if not any(
                inst.sync_info is not None
                and len(inst.sync_info.on_wait) > max_waits
                for inst in insts
            ):
                continue
            new = []
            for inst in insts:
                si = inst.sync_info
                waits = list(si.on_wait) if si is not None and si.on_wait else []
                if len(waits) > max_waits:
                    head, tail = waits[:-max_waits], waits[-max_waits:]
                    for w in head:
                        _wsplit_counter[0] += 1
                        nop = mybir.InstNoOp(
                            name=f"wsplit-{_wsplit_counter[0]}", ins=[], outs=[]
                        )
                        nop.engine = inst.engine
                        nop.sync_info = bass_rust.SyncInfo(on_wait=[w], on_update=[])
                        new.append(nop)
                        n_split += 1
                    inst.sync_info = bass_rust.SyncInfo(
                        on_wait=tail, on_update=list(si.on_update)
                    )
                new.append(inst)
            insts[:] = new
    return n_split


_orig_tile_exit = tile_mod.TileContext.__exit__


def _patched_tile_exit(self, *args, **kwargs):
    ret = _orig_tile_exit(self, *args, **kwargs)
    _split_excess_waits(self.nc)
    return ret


if getattr(tile_mod.TileContext, "_attn_patch", None) is None:
    tile_mod.TileContext._drain_and_barrier = _patched_drain_and_barrier
    tile_mod.TileContext.__exit__ = _patched_tile_exit
    tile_mod.TileContext._attn_patch = True

# ---------------------------------------------------------------------------
# Program constants
# ---------------------------------------------------------------------------
f32 = mybir.dt.float32
bf16 = mybir.dt.bfloat16
AF = mybir.ActivationFunctionType
ALU = mybir.AluOpType

B, L, D = 4, 1024, 1024
CH = 512          # channels per core (8 heads x dh=64)
DC = 8            # d (contraction) chunks of 128
KC = 8            # key-position chunks of 128
LC = 8            # l (query/row) chunks of 128
N_CORES = 8
SCALE = 0.125     # 1/sqrt(dh)
MASK_BIAS = -18.0  # added to s/8 for masked keys pre-exp


def build_program():
    nc = bass.Bass(trn_type="TRN2", target_bir_lowering=False, debug=False)

    qT_d = nc.dram_tensor("qT", [D, L], bf16, kind="ExternalInput").ap()
    kT_d = nc.dram_tensor("kT", [D, L], bf16, kind="ExternalInput").ap()
    vT_d = nc.dram_tensor("vT", [D, L], bf16, kind="ExternalInput").ap()
    wq_d = nc.dram_tensor("wq", [D, CH], bf16, kind="ExternalInput").ap()
    wk_d = nc.dram_tensor("wk", [D, CH], bf16, kind="ExternalInput").ap()
    wv_d = nc.dram_tensor("wv", [D, CH], bf16, kind="ExternalInput").ap()
    wo_d = nc.dram_tensor("wo", [CH, D], bf16, kind="ExternalInput").ap()
    kmb_d = nc.dram_tensor("kmb", [128, KC], bf16, kind="ExternalInput").ap()
    mb_d = nc.dram_tensor("mb", [128, KC], f32, kind="ExternalInput").ap()
    qm_d = nc.dram_tensor("qm", [128, LC], f32, kind="ExternalInput").ap()
    out_d = nc.dram_tensor("out", [L, D], bf16, kind="ExternalOutput").ap()
    scr_d = nc.dram_tensor("scr", [4, 4, 512], f32).ap()

    with tile.TileContext(nc) as tc:
        with (
            tc.tile_pool(name="persist", bufs=1) as pers,
            tc.tile_pool(name="work", bufs=3) as work,
            tc.tile_pool(name="etp", bufs=6) as etp,
            tc.tile_pool(name="normp", bufs=6) as normp,
        ):
            # ---- persistent SBUF tiles ----
            wo_t = pers.tile([128, 4 * 1024], bf16, tag="wo")
            kmb_t = pers.tile([128, KC], bf16, tag="kmb")
            mb_t = pers.tile([128, KC], f32, tag="mb")
            qm_t = pers.tile([128, LC], f32, tag="qm")
            QT_t = pers.tile([128, 4 * 1024], bf16, tag="QT")
            KT_t = pers.tile([128, 4 * 1024], bf16, tag="KT")
            V_t = pers.tile([128, KC * 512], bf16, tag="V")
            vT_sb = pers.tile([128, DC * 1024], bf16, tag="vTsb")
            OT_ts = [pers.tile([128, 1024], bf16, tag=f"OT{i}", name=f"OT{i}")
                     for i in range(4)]

            nc.gpsimd.dma_start(kmb_t[:], kmb_d)
            nc.gpsimd.dma_start(mb_t[:], mb_d)
            nc.gpsimd.dma_start(qm_t[:], qm_d)

            # pre-trigger the exp table load while the PE is still in the
            # DMA-bound startup window (saves ~2.7us at first real exp)
            warm_t = pers.tile([1, 8], f32, tag="warm")
            nc.scalar.activation(warm_t[:], qm_t[0:1, 0:8], AF.Exp)

            def load_w(pool, dram):
                t = pool.tile([128, DC * 512], bf16,
                              tag=dram.tensor.name + "_t",
                              name=dram.tensor.name + "_t")
                return t

            from contextlib import ExitStack
            _es = ExitStack()
            wpool = _es.enter_context(tc.tile_pool(name="wpool", bufs=1))
            kpin = _es.enter_context(tc.tile_pool(name="kpin", bufs=8))
            qpin = _es.enter_context(tc.tile_pool(name="qpin", bufs=8))

            wk_t = load_w(wpool, wk_d)
            wq_t = load_w(wpool, wq_d)
            wv_t = load_w(wpool, wv_d)
            wv_s3 = wv_d.rearrange("(d p) n -> d p n", p=128)
            wk_s3 = wk_d.rearrange("(d p) n -> d p n", p=128)
            wq_s3 = wq_d.rearrange("(d p) n -> d p n", p=128)
            v3 = vT_d.rearrange("(d p) l -> d p l", p=128)
            k3 = kT_d.rearrange("(d p) l -> d p l", p=128)
            q3 = qT_d.rearrange("(d p) l -> d p l", p=128)

            # input DMA stream: k then q (attention-critical), then v
            k_cs, q_cs = [], []
            for d in range(DC):
                nc.sync.dma_start(wk_t[:, d * 512:(d + 1) * 512], wk_s3[d])
                x_c = kpin.tile([128, L], bf16, tag="kpin", name=f"kc_{d}")
                nc.sync.dma_start(x_c[:], k3[d])
                k_cs.append(x_c)
            for d in range(DC):
                nc.sync.dma_start(wq_t[:, d * 512:(d + 1) * 512], wq_s3[d])
                x_c = qpin.tile([128, L], bf16, tag="qpin", name=f"qc_{d}")
                nc.sync.dma_start(x_c[:], q3[d])
                q_cs.append(x_c)
            for d in range(DC):
                nc.sync.dma_start(wv_t[:, d * 512:(d + 1) * 512], wv_s3[d])
                nc.sync.dma_start(vT_sb[:, d * 1024:(d + 1) * 1024], v3[d])

            # one projection unit: psum-accumulate 8 d-chunks for (ci, lh)
            # and copy the [128,512] result into KT_t/QT_t
            def proj_unit(pool, w_t, x_cs, dst, ci, lh, eng):
                ps = pool.tile([128, 512], f32, tag="pj",
                               name=f"pj_{dst.tensor.name}_{ci}_{lh}")
                for d in range(DC):
                    nc.tensor.matmul(
                        ps[:],
                        w_t[:, d * 512 + ci * 128: d * 512 + (ci + 1) * 128],
                        x_cs[d][:, lh * 512:(lh + 1) * 512],
                        start=(d == 0), stop=(d == DC - 1),
                    )
                dsl = dst[:, ci * 1024 + lh * 512: ci * 1024 + (lh + 1) * 512]
                if eng == 0:
                    nc.vector.tensor_copy(dsl, ps[:])
                else:
                    nc.scalar.activation(dsl, ps[:], AF.Copy)

            # ci0/ci1 of K and Q before attention (DMA-paced)
            with tc.tile_pool(name="psPJ", bufs=4, space="PSUM") as psPJ:
                # ~3.4us of junk matmuls on uninitialized SBUF: trips the
                # HAM activity monitor so the real projection matmuls run
                # at 2.4 GHz instead of the cold 1.2 GHz default
                with tc.tile_pool(name="warmP", bufs=1,
                                  space="PSUM") as warmP:
                    wps = warmP.tile([128, 512], f32, tag="wps")
                    for i in range(8):
                        nc.tensor.matmul(
                            wps[:], vT_sb[:, 0:128], vT_sb[:, 0:512],
                            start=True, stop=True,
                        )
                for ci in range(2):
                    for lh in range(2):
                        proj_unit(psPJ, wk_t, k_cs, KT_t, ci, lh, lh)
                for ci in range(2):
                    for lh in range(2):
                        proj_unit(psPJ, wq_t, q_cs, QT_t, ci, lh, lh)

            # ---- attention: head pairs with PE tiling ----
            # per (pair, ki): row-tiled concurrent scores for heads A/B into
            # one [128,1024] st tile (A cols 0:512, B cols 512:1024), one exp
            # with per-partition mask bias, col-tiled concurrent PV into one
            # u bank (A -> partitions 0:64, B -> 64:128), and 4-way col-tiled
            # M=1 denominator matmuls into strips of one D bank.
            usbs = {}   # (p, qh) -> stashed numerator SBUF tile [128, 512]
            us = {}     # p -> [u_q0, u_q1] PSUM numerator banks
            dbs = {}    # p -> denominator PSUM bank

            def emit_recip(p):
                db = dbs[p]
                rr = normp.tile([97, 512], f32, tag="rr", name=f"rr_{p}")
                nc.vector.reciprocal(rr[:], db[0:97, :])
                for si in range(4):
                    nc.sync.dma_start(scr_d[p, si, :],
                                      rr[32 * si:32 * si + 1, :])

            def attn_all(stP, psU, psD, slot_end=None):
                """Software-pipelined attention over all pairs: slot s emits
                scores+exp for (p,ki)=s and PV+denominators for slot s-1, so
                the PE never waits on the ACT exp and vice versa. slot_end(s)
                fills spare PE time with aux work (V-proj, ci2/3 proj)."""
                slots = [(p, ki) for p in range(4) for ki in range(KC)]
                ets = {}

                def emit_scores(p, ki):
                    co = p * 1024
                    ksl = slice(co + ki * 128, co + (ki + 1) * 128)
                    for qh in range(2):
                        qsl = slice(co + qh * 512, co + (qh + 1) * 512)
                        st = stP.tile([128, 1024], f32, tag="st",
                                      name=f"st_{p}_{ki}_{qh}")
                        nc.tensor.matmul(
                            st[:, 0:512], KT_t[0:64, ksl], QT_t[0:64, qsl],
                            start=True, stop=True,
                        )
                        nc.tensor.matmul(
                            st[:, 512:1024], KT_t[64:128, ksl],
                            QT_t[64:128, qsl],
                            start=True, stop=True,
                        )
                        et = etp.tile([128, 1024], bf16, tag="et",
                                      name=f"et_{p}_{ki}_{qh}")
                        nc.scalar.activation(et[:], st[:], AF.Exp,
                                             scale=SCALE,
                                             bias=mb_t[:, ki:ki + 1])
                        ets[(p, ki, qh)] = et

                def emit_pv_d(p, ki):
                    hA, hB = 2 * p, 2 * p + 1
                    u = us[p]
                    db = dbs[p]
                    for qh in range(2):
                        et = ets[(p, ki, qh)]
                        nc.tensor.matmul(
                            u[qh][0:64, :],
                            V_t[:, ki * 512 + hA * 64: ki * 512 + hA * 64 + 64],
                            et[:, 0:512],
                            start=(ki == 0), stop=(ki == KC - 1),
                        )
                        nc.tensor.matmul(
                            u[qh][64:128, :],
                            V_t[:, ki * 512 + hB * 64: ki * 512 + hB * 64 + 64],
                            et[:, 512:1024],
                            start=(ki == 0), stop=(ki == KC - 1),
                        )
                    # all 4 denominator matmuls adjacent -> 4-way col tiling
                    for qh in range(2):
                        et = ets[(p, ki, qh)]
                        for a in range(2):
                            si = 2 * qh + a
                            nc.tensor.matmul(
                                db[32 * si:32 * si + 1, :],
                                kmb_t[:, ki:ki + 1],
                                et[:, a * 512:(a + 1) * 512],
                                start=(ki == 0), stop=(ki == KC - 1),
                                tile_position=(0, 32 * si),
                            )

                def finish_pair(p):
                    # stash numerators to SBUF (frees u banks); reciprocal
                    # + DRAM round-trip for the partition-broadcast. Pair
                    # 3's reciprocal is hoisted into the outproj prologue
                    # so it overlaps the first outproj matmuls instead of
                    # serializing them through the pool-exit bank WAR.
                    u = us[p]
                    for qh in range(2):
                        usb = normp.tile([128, 512], f32, tag="usb",
                                         name=f"usb_{p}_{qh}")
                        nc.vector.tensor_copy(usb[:], u[qh][:])
                        usbs[(p, qh)] = usb
                    if p < 3:
                        emit_recip(p)

                for s in range(len(slots) + 1):
                    cur = slots[s] if s < len(slots) else None
                    if cur is not None:
                        p, ki = cur
                        if ki == 0:
                            us[p] = [psU.tile([128, 512], f32, tag="u",
                                              name=f"u_{p}_{q}")
                                     for q in range(2)]
                            dbs[p] = psD.tile([128, 512], f32, tag="db",
                                              name=f"db_{p}")
                        if (p, ki) == (1, 0):
                            wo3 = wo_d.rearrange("(c p) n -> c p n", p=128)
                            for ci in range(4):
                                nc.gpsimd.dma_start(
                                    wo_t[:, ci * 1024:(ci + 1) * 1024],
                                    wo3[ci])
                        emit_scores(p, ki)
                    if s > 0:
                        pp, pki = slots[s - 1]
                        emit_pv_d(pp, pki)
                        if pki == KC - 1:
                            finish_pair(pp)
                        if pki == 3 and pp >= 1:
                            norm_apply(pp - 1)
                    if slot_end is not None:
                        slot_end(s)

            def norm_apply(p):
                # OT_ts[p][:, qh*512:+512] = usb * (1/D) with the recip row
                # broadcast across partitions via DRAM round trip
                for qh in range(2):
                    rb = work.tile([128, 512], f32, tag="rb",
                                   name=f"rb_{p}_{qh}")
                    for a in range(2):
                        src = scr_d[p, 2 * qh + a, :]
                        bcast = bass.AP(
                            tensor=src.tensor, offset=src.offset,
                            ap=[[0, 64]] + list(src.ap),
                        )
                        nc.gpsimd.dma_start(rb[64 * a:64 * a + 64, :], bcast)
                    nc.gpsimd.tensor_tensor(
                        OT_ts[p][:, qh * 512:(qh + 1) * 512],
                        usbs[(p, qh)][:], rb[:], ALU.mult,
                    )

            psD_cm = tc.tile_pool(name="psD", bufs=1, space="PSUM")
            psD = psD_cm.__enter__()
            with tc.tile_pool(name="psU", bufs=2, space="PSUM") as psU, \
                 tc.tile_pool(name="stP", bufs=2, space="PSUM") as stP, \
                 tc.tile_pool(name="auxP", bufs=1, space="PSUM") as auxP:
                def vproj_unit(ki):
                    psv = auxP.tile([128, 512], f32, tag="pj",
                                    name=f"psv_{ki}")
                    for d in range(DC):
                        nc.tensor.matmul(
                            psv[:],
                            vT_sb[:, d * 1024 + ki * 128:
                                  d * 1024 + (ki + 1) * 128],
                            wv_t[:, d * 512:(d + 1) * 512],
                            start=(d == 0), stop=(d == DC - 1),
                        )
                    nc.vector.tensor_copy(
                        V_t[:, ki * 512:(ki + 1) * 512], psv[:])

                # aux schedule: pair0 slot-ends project V chunk ki (consumed
                # by PV one slot later); pair1/pair2 slot-ends run the ci2/
                # ci3 K,Q projection units (DVE copies only - ACT is busy)
                def slot_end(s):
                    if s < KC:
                        vproj_unit(s)
                    elif KC <= s < 2 * KC and s % 2 == 0:
                        j = (s - KC) // 2
                        w_t, x_cs, dst = [(wk_t, k_cs, KT_t),
                                          (wq_t, q_cs, QT_t)][j // 2]
                        proj_unit(auxP, w_t, x_cs, dst, 2, j % 2, 0)
                    elif 2 * KC <= s < 3 * KC and s % 2 == 0:
                        j = (s - 2 * KC) // 2
                        w_t, x_cs, dst = [(wk_t, k_cs, KT_t),
                                          (wq_t, q_cs, QT_t)][j // 2]
                        proj_unit(auxP, w_t, x_cs, dst, 3, j % 2, 0)

                attn_all(stP, psU, psD, slot_end=slot_end)

            # ---- output projection ----
            with tc.tile_pool(name="psPP", bufs=7, space="PSUM") as psPP:
                def outproj_mms(po, li, oh, hps, first, last):
                    for hp in hps:
                        nc.tensor.matmul(
                            po[:],
                            OT_ts[hp][:, li * 128:(li + 1) * 128],
                            wo_t[:, hp * 1024 + oh * 512:
                                 hp * 1024 + (oh + 1) * 512],
                            start=(first and hp == hps[0]),
                            stop=(last and hp == hps[-1]),
                        )

                def finalize(po, li, oh):
                    ob = work.tile([128, 512], bf16, tag="ob",
                                   name=f"ob_{li}_{oh}")
                    if (li * 2 + oh) % 2 == 0:
                        nc.scalar.activation(
                            ob[:], po[:], AF.Copy,
                            scale=qm_t[:, li:li + 1],
                        )
                    else:
                        nc.vector.tensor_scalar(
                            ob[:], po[:], qm_t[:, li:li + 1],
                            None, ALU.mult,
                        )
                    nc.sync.dma_start(
                        out_d[li * 128:(li + 1) * 128,
                              oh * 512:(oh + 1) * 512],
                        ob[:],
                    )

                blocks = [(li, oh) for li in range(LC) for oh in range(2)]
                pos = {}
                # pair-3 reciprocal + normalize land while the hp0-2
                # matmuls of the first 7 blocks run; hp3 matmuls follow.
                emit_recip(3)
                for li, oh in blocks[:7]:
                    po = psPP.tile([128, 512], f32, tag="pp",
                                   name=f"po_{li}_{oh}")
                    outproj_mms(po, li, oh, [0, 1, 2], True, False)
                    pos[(li, oh)] = po
                norm_apply(3)
                for li, oh in blocks[:7]:
                    po = pos[(li, oh)]
                    outproj_mms(po, li, oh, [3], False, True)
                    finalize(po, li, oh)
                for li, oh in blocks[7:]:
                    po = psPP.tile([128, 512], f32, tag="pp",
                                   name=f"po_{li}_{oh}")
                    outproj_mms(po, li, oh, [0, 1, 2, 3], True, True)
                    finalize(po, li, oh)
            psD_cm.__exit__(None, None, None)
            _es.close()
    return nc


_cache = {}


def _get_program():
    if "nc" not in _cache:
        _cache["nc"] = build_program()
    return _cache["nc"]


BF16_NP = mybir.dt.np(bf16)


def build_in_maps(query, key, value, query_mask, key_mask, Wq, Wk, Wv, Wo):
    query = np.asarray(query, dtype=np.float32)
    key = np.asarray(key, dtype=np.float32)
    value = np.asarray(value, dtype=np.float32)
    Wq = np.asarray(Wq, dtype=np.float32)
    Wk = np.asarray(Wk, dtype=np.float32)
    Wv = np.asarray(Wv, dtype=np.float32)
    Wo = np.asarray(Wo, dtype=np.float32)

    qT = [np.ascontiguousarray(query[b].T).astype(BF16_NP) for b in range(B)]
    kT = [np.ascontiguousarray(key[b].T).astype(BF16_NP) for b in range(B)]
    vT = [np.ascontiguousarray(value[b].T).astype(BF16_NP) for b in range(B)]
    kmf = [key_mask[b].astype(np.float32).reshape(KC, 128).T for b in range(B)]
    kmb = [np.ascontiguousarray(m).astype(BF16_NP) for m in kmf]
    # bias = 0 for kept keys, MASK_BIAS (-18) for masked keys
    mb = [np.ascontiguousarray((1.0 - m) * MASK_BIAS) for m in kmf]
    qm = [
        np.ascontiguousarray(query_mask[b].astype(np.float32).reshape(LC, 128).T)
        for b in range(B)
    ]
    wq_g = [np.ascontiguousarray(Wq[:, g * CH:(g + 1) * CH]).astype(BF16_NP)
            for g in range(2)]
    wk_g = [np.ascontiguousarray(Wk[:, g * CH:(g + 1) * CH]).astype(BF16_NP)
            for g in range(2)]
    wv_g = [np.ascontiguousarray(Wv[:, g * CH:(g + 1) * CH]).astype(BF16_NP)
            for g in range(2)]
    wo_g = [np.ascontiguousarray(Wo[g * CH:(g + 1) * CH, :]).astype(BF16_NP)
            for g in range(2)]

    in_maps = []
    for c in range(N_CORES):
        b, g = c // 2, c % 2
        in_maps.append({
            "qT": qT[b], "kT": kT[b], "vT": vT[b],
            "wq": wq_g[g], "wk": wk_g[g], "wv": wv_g[g], "wo": wo_g[g],
            "kmb": kmb[b], "mb": mb[b], "qm": qm[b],
        })
    return in_maps


def kernel(query, key, value, query_mask, key_mask, Wq, Wk, Wv, Wo):
    nc = _get_program()
    in_maps = build_in_maps(query, key, value, query_mask, key_mask,
                            Wq, Wk, Wv, Wo)
    res = run_bass_kernel_spmd(nc, in_maps, list(range(N_CORES)))
    out = np.empty((B, L, D), dtype=np.float32)
    for b in range(B):
        out[b] = (res.results[2 * b]["out"].astype(np.float32)
                  + res.results[2 * b + 1]["out"].astype(np.float32))
    return out
